# revision 5
# baseline (speedup 1.0000x reference)
"""Trainium2 Bass kernel: dual-attention transformer block, fp8 DoubleRow.

Reference semantics (per batch element b):
    q/k/v = x_b @ sa_w{q,k,v} + sa_b{q,k,v}
    sa    = softmax(q k^T / sqrt(DB)) v @ sa_wo + sa_bo
    x_b1  = x_b + sa
    q     = x_a @ ca_wq + ca_bq ; k/v = x_b1 @ ca_w{k,v} + ca_b{k,v}
    out   = x_b1 + softmax(q k^T / sqrt(DA)) v @ ca_wo + ca_bo

Sharding: data-parallel over batch - 8 batch elements, one per NeuronCore.

All matmuls run in fp8e4 with MatmulPerfMode.DoubleRow (K=256 per instruction,
~2.5x bf16 throughput measured on HW).  Numerics plan:
  - weights scaled x32 on host (fp8e4 normal range starts at 2^-6); exact
    powers of two unwound via the exp scale (1/(1024*sqrt(D))), the AV
    output scale (1/64) and the softmax-reciprocal path.
  - scores are computed TRANSPOSED (s^T[j,i] = k_j . q_i) so the exp output
    lands directly in the [key, query] layout the AV matmul needs: no N x N
    transpose, no N x N normalization pass.  exp() output goes straight to
    fp8 (unnormalized weights, range ~[0.05, 20], fp8e4 max 240).
  - softmax sums (over the partition axis) via a ones-vector DoubleRow
    matmul -> [1, n] psum; a tiny DRAM round-trip re-lays [1, n] as
    [128, n/128] so the reciprocal becomes a per-partition scale applied at
    the out-projection psum (out rows = queries).
  - k-bias is softmax-invariant (dropped); v-bias/out-bias folded on host
    into the residual; q-bias added on device at the q psum->fp8 cast.
  - residual stream bf16 (x_b+b_eff as input, xb1 roundtrip); final output
    written f32.
"""

import math
import os
from contextlib import ExitStack

import numpy as np
import ml_dtypes

import concourse.bass as bass
import concourse.mybir as mybir
import concourse.tile as tile
from concourse import bacc
from concourse.bass_utils import run_bass_kernel_spmd

P = 128
F32 = mybir.dt.float32
BF16 = mybir.dt.bfloat16
F8 = mybir.dt.float8e4
AF = mybir.ActivationFunctionType
ALU = mybir.AluOpType
DR = mybir.MatmulPerfMode.DoubleRow

B_FULL, N_FULL, DA_FULL, DB_FULL = 8, 2048, 768, 1024

WS = 32.0        # host weight scale (all projection weights)
ONES_VAL = 16.0  # sums = 16*sum(e);  16 = (WS_v * WS_o) / AT_DOWN
AT_DOWN = 64.0   # AV psum -> fp8 downscale


def build_block(tc, outs, ins, n, da, db):
    """Emit the dual-attention block into TileContext `tc`.

    ins (all DRAM APs, fp8 tensors pre-scaled/pre-laid-out on host):
      xbT [P,KB,n] f8, xaT [P,KA,n] f8, xbpb [n,db] bf16,
      sa_wq/sa_wk/sa_wv/sa_wo [P,KB,db] f8, ca_wq [P,KA,db] f8,
      ca_wk/ca_wv/ca_wo [P,KB,db] f8, bq_sa/bq_ca [P,KB] f32, ones [P,2,16] f8
    outs: out [n,db] f32
    """
    nc = tc.nc
    KB, KA, NI = db // P, da // P, n // P
    SB = min(2048, n)          # attention superblock / psA span
    NSB = n // SB
    SPB = SB // P              # i-blocks per superblock
    IC = min(512, SB)          # moving chunk (DoubleRow out free dim)
    ICS = SB // IC
    ECW = min(IC, db)          # out-proj / v-proj free chunk
    NEC = db // ECW
    MC = min(256, n)           # xb1 transpose chunk
    NJP = NI // 2              # j-tile pairs (AV / sums contraction)
    assert KB % 2 == 0 and KA % 2 == 0 and NI % 2 == 0 and 2 * db <= SB

    sc_sa = 1.0 / (WS * WS * math.sqrt(float(db)))
    sc_ca = 1.0 / (WS * WS * math.sqrt(float(da)))

    ctx = ExitStack()
    with ctx:
        sp = ctx.enter_context(tc.tile_pool(name="sp", bufs=1))
        pp = ctx.enter_context(tc.tile_pool(name="pp", bufs=1, space="PSUM"))
        dp = ctx.enter_context(tc.tile_pool(name="dp", bufs=1, space="DRAM"))

        xb1b_d = dp.tile([n, db], BF16, tag="xb1b")

        # ---- persistent SBUF ----
        kT = sp.tile([P, KB, n], F8, tag="kT")      # k^T [feat, seq]
        qT = sp.tile([P, KB, n], F8, tag="qT")      # q^T (SA then CA)
        v_sb = sp.tile([P, NI, db], F8, tag="v")    # v   [seq, feat]
        xbT = sp.tile([P, KB, n], F8, tag="xbT")
        xaT = sp.tile([P, KA, n], F8, tag="xaT")
        x1T = sp.tile([P, KB, n], F8, tag="x1T")    # xb1^T fp8 for CA k/v
        bqs = sp.tile([P, KB], F32, tag="bq", bufs=2)
        bqc = sp.tile([P, KB], F32, tag="bq", bufs=2)
        ones = sp.tile([P, 2, 16], F8, tag="ones")
        nc.sync.dma_start(bqs[:], ins["bq_sa"][:])
        nc.sync.dma_start(bqc[:], ins["bq_ca"][:])
        nc.sync.dma_start(ones[:], ins["ones"][:])

        def load_w(name, ktiles):
            # two half-loads so consumers of the first k-tiles start early
            wt = sp.tile([P, KB, db], F8, tag="w", bufs=3)
            h = max(1, ktiles // 2)
            nc.sync.dma_start(wt[:, :h, :], ins[name][:, :h, :])
            if h < ktiles:
                nc.sync.dma_start(wt[:, h:ktiles, :], ins[name][:, h:ktiles, :])
            return wt

        def proj_qk(xT_sb, ktiles, targets):
            # yT[f, i] = sum_k w[k, f] xT[k, i]  (+ per-partition bias), fp8 out.
            # targets: list of (w_sb, bias_or_None, dst [P,KB,n])
            nkp = ktiles // 2
            for ih in range(n // SB):
                for fb in range(KB):
                    for (w_sb, bias, dst) in targets:
                        ps = pp.tile([P, SB], F32, tag="psA", bufs=2)
                        for kp in range(nkp):
                            for ic in range(ICS):
                                nc.tensor.matmul(
                                    ps[:, ic * IC:(ic + 1) * IC],
                                    w_sb[:, 2 * kp:2 * kp + 2, fb * P:(fb + 1) * P],
                                    xT_sb[:, 2 * kp:2 * kp + 2,
                                          ih * SB + ic * IC:ih * SB + (ic + 1) * IC],
                                    start=(kp == 0), stop=(kp == nkp - 1),
                                    perf_mode=DR,
                                )
                        d = dst[:, fb, ih * SB:(ih + 1) * SB]
                        b = 0.0 if bias is None else bias[:, fb:fb + 1]
                        nc.scalar.activation(d, ps[:], AF.Identity, bias=b)

        def proj_v(xT_sb, ktiles, w_sb, mb_range):
            # v[m, e] = sum_k xT[k, m] w[k, e], fp8 out (natural layout).
            # mb pairs share one [P, SB] psum tile (2*db <= SB).
            nkp = ktiles // 2
            for mbp in mb_range[::2]:
                ps = pp.tile([P, SB], F32, tag="psA", bufs=2)
                for off in range(2):
                    for kp in range(nkp):
                        for ec in range(NEC):
                            nc.tensor.matmul(
                                ps[:, (off * NEC + ec) * ECW:
                                   (off * NEC + ec + 1) * ECW],
                                xT_sb[:, 2 * kp:2 * kp + 2,
                                      (mbp + off) * P:(mbp + off + 1) * P],
                                w_sb[:, 2 * kp:2 * kp + 2, ec * ECW:(ec + 1) * ECW],
                                start=(kp == 0), stop=(kp == nkp - 1),
                                perf_mode=DR,
                            )
                nc.vector.tensor_copy(v_sb[:, mbp:mbp + 2, :], ps[:, :2 * db])

        def attention(scale, wo_sb, resid_dram, writer, hooks):
            # hooks: {emission_point_name: fn()} to interleave other phases
            nkp = KB // 2
            ndp = KB // 2

            def scores(sb, wt_t):
                for jb in range(NI):
                    ps = pp.tile([P, SB], F32, tag="psA", bufs=2)
                    for kp in range(nkp):
                        for ic in range(ICS):
                            nc.tensor.matmul(
                                ps[:, ic * IC:(ic + 1) * IC],
                                kT[:, 2 * kp:2 * kp + 2, jb * P:(jb + 1) * P],
                                qT[:, 2 * kp:2 * kp + 2,
                                   sb * SB + ic * IC:sb * SB + (ic + 1) * IC],
                                start=(kp == 0), stop=(kp == nkp - 1),
                                perf_mode=DR,
                            )
                    nc.scalar.activation(wt_t[:, jb, :], ps[:], AF.Exp,
                                         bias=0.0, scale=scale)

            def sums(sb, wt_t, rr_t):
                sums_t = dp.tile([SB], F32, tag="sums", bufs=4)
                pss = pp.tile([P, SB], F32, tag="psA", bufs=2, name="ps_sum")
                for ic in range(ICS):
                    for jp in range(NJP):
                        nc.tensor.matmul(
                            pss[0:1, ic * IC:(ic + 1) * IC],
                            ones[:, :, 0:1],
                            wt_t[:, 2 * jp:2 * jp + 2, ic * IC:(ic + 1) * IC],
                            start=(jp == 0), stop=(jp == NJP - 1),
                            perf_mode=DR,
                        )
                ssb = sp.tile([1, SB], F32, tag="ss", bufs=1)
                nc.vector.tensor_copy(ssb[:], pss[0:1, :])
                nc.sync.dma_start(sums_t[:], ssb[:])
                rrb = sp.tile([P, SPB], F32, tag="rrb", bufs=2)
                nc.sync.dma_start(rrb[:], sums_t.rearrange("(t p) -> p t", p=P))
                nc.vector.reciprocal(rr_t[:], rrb[:])

            def av(sb, wt_t, at_t):
                for dt in range(KB):
                    pss = pp.tile([P, SB], F32, tag="psA", bufs=2, name="ps_av")
                    for jp in range(NJP):
                        for ic in range(ICS):
                            nc.tensor.matmul(
                                pss[:, ic * IC:(ic + 1) * IC],
                                v_sb[:, 2 * jp:2 * jp + 2, dt * P:(dt + 1) * P],
                                wt_t[:, 2 * jp:2 * jp + 2, ic * IC:(ic + 1) * IC],
                                start=(jp == 0), stop=(jp == NJP - 1),
                                perf_mode=DR,
                            )
                    nc.vector.tensor_scalar_mul(at_t[:, dt, :], pss[:],
                                                1.0 / AT_DOWN)

            def outproj(sb, at_t, rr_t, rx_list):
                for q3p in range(0, SPB, 2):
                    pso = pp.tile([P, SB], F32, tag="psA", bufs=2, name="ps_op")
                    for off in range(2):
                        q3 = q3p + off
                        for dp_ in range(ndp):
                            for ec in range(NEC):
                                sl = (off * NEC + ec) * ECW
                                nc.tensor.matmul(
                                    pso[:, sl:sl + ECW],
                                    at_t[:, 2 * dp_:2 * dp_ + 2,
                                         q3 * P:(q3 + 1) * P],
                                    wo_sb[:, 2 * dp_:2 * dp_ + 2,
                                          ec * ECW:(ec + 1) * ECW],
                                    start=(dp_ == 0), stop=(dp_ == ndp - 1),
                                    perf_mode=DR,
                                )
                    for off in range(2):
                        q3 = q3p + off
                        ib = sb * SPB + q3
                        ro = sp.tile([P, db], writer.dtype, tag=writer.tag,
                                     bufs=2, name="ro")
                        nc.scalar.activation(
                            ro[:], pso[:, off * db:(off + 1) * db], AF.Identity,
                            bias=0.0, scale=rr_t[:, q3:q3 + 1])
                        nc.vector.tensor_tensor(ro[:], ro[:], rx_list[q3][:],
                                                ALU.add)
                        writer(ib, ro)
                    if "after_pair" in hooks:
                        hooks["after_pair"](sb * SPB + q3p)

            def rx_load(sb):
                lst = []
                for q3 in range(SPB):
                    ib = sb * SPB + q3
                    rx = sp.tile([P, db], BF16, tag="rx", bufs=min(SPB, 6))
                    nc.sync.dma_start(rx[:], resid_dram[ib * P:(ib + 1) * P, :])
                    lst.append(rx)
                return lst

            def tail(sb):
                # sums -> (hook) -> rx prefetch -> AV -> out-proj for sb
                sums(sb, wt[sb], rr[sb])
                if sb == 0 and "after_sums0" in hooks:
                    hooks["after_sums0"]()
                rx = rx_load(sb)
                at_t = sp.tile([P, KB, SB], F8, tag="at", bufs=min(NSB, 2), name="at_t")
                av(sb, wt[sb], at_t)
                outproj(sb, at_t, rr[sb], rx)
                if "after_op" in hooks:
                    hooks["after_op"](sb)

            wt, rr = {}, {}
            for sb in range(NSB):
                wt[sb] = sp.tile([P, NI, SB], F8, tag="wt", bufs=min(NSB, 2), name="wt_t")
                rr[sb] = sp.tile([P, SPB], F32, tag="rr", bufs=2, name="rr_t")
                scores(sb, wt[sb])
                if sb >= 1:
                    tail(sb - 1)
            tail(NSB - 1)

        def sa_writer(ib, ro):
            nc.scalar.dma_start(xb1b_d[ib * P:(ib + 1) * P, :], ro[:])
        sa_writer.dtype, sa_writer.tag = BF16, "roA"

        def ca_writer(ib, ro):
            nc.scalar.dma_start(outs["out"][ib * P:(ib + 1) * P, :], ro[:])
        ca_writer.dtype, ca_writer.tag = F32, "roB"

        # ===================== self-attention =====================
        wq = load_w("sa_wq", KB)
        wk = load_w("sa_wk", KB)
        nc.sync.dma_start(xbT[:], ins["xbT"][:])
        proj_qk(xbT, KB, [(wq, bqs, qT), (wk, None, kT)])
        wv = load_w("sa_wv", KB)
        proj_v(xbT, KB, wv, range(NI))
        wo = load_w("sa_wo", KB)

        # CA q (from x_a) is emitted inside the SA attention via hook: it
        # fills the PE while SA's softmax tail / rr round-trips complete.
        nc.sync.dma_start(xaT[:, :KA, :], ins["xaT"][:])
        wq2 = load_w("ca_wq", KA)

        def emit_ca_q():
            proj_qk(xaT, KA, [(wq2, bqc, qT)])

        def emit_xpose(ibp):
            # transpose the xb1 rows finished by out-proj pair starting at ibp
            r0 = ibp * P
            for mcc in range(r0 // MC, (r0 + 2 * P) // MC):
                xTb = sp.tile([P, KB, MC], BF16, tag="xtb", bufs=2)
                nc.sync.dma_start_transpose(
                    xTb[:], xb1b_d[mcc * MC:(mcc + 1) * MC, :])
                nc.vector.tensor_copy(x1T[:, :, mcc * MC:(mcc + 1) * MC], xTb[:])

        attention(sc_sa, wo, ins["xbpb"], sa_writer,
                  {"after_sums0": emit_ca_q, "after_pair": emit_xpose})

        # ===================== cross-attention =====================
        wv2 = load_w("ca_wv", KB)
        proj_v(x1T, KB, wv2, range(NI))
        wk2 = load_w("ca_wk", KB)
        proj_qk(x1T, KB, [(wk2, None, kT)])
        wo2 = load_w("ca_wo", KB)
        attention(sc_ca, wo2, xb1b_d, ca_writer, {})


def build_program(n=N_FULL, da=DA_FULL, db=DB_FULL, repeat=1):
    nc = bacc.Bacc("TRN2", target_bir_lowering=False, debug=False, enable_asserts=False)
    KB, KA = db // P, da // P
    ins = {
        "xbT": nc.dram_tensor("xbT", [P, KB, n], F8, kind="ExternalInput").ap(),
        "xaT": nc.dram_tensor("xaT", [P, KA, n], F8, kind="ExternalInput").ap(),
        "xbpb": nc.dram_tensor("xbpb", [n, db], BF16, kind="ExternalInput").ap(),
        "bq_sa": nc.dram_tensor("bq_sa", [P, KB], F32, kind="ExternalInput").ap(),
        "bq_ca": nc.dram_tensor("bq_ca", [P, KB], F32, kind="ExternalInput").ap(),
        "ones": nc.dram_tensor("ones", [P, 2, 16], F8, kind="ExternalInput").ap(),
    }
    for nm in ("sa_wq", "sa_wk", "sa_wv", "sa_wo", "ca_wk", "ca_wv", "ca_wo"):
        ins[nm] = nc.dram_tensor(nm, [P, KB, db], F8, kind="ExternalInput").ap()
    ins["ca_wq"] = nc.dram_tensor("ca_wq", [P, KA, db], F8, kind="ExternalInput").ap()
    outs = {"out": nc.dram_tensor("out", [n, db], F32, kind="ExternalOutput").ap()}
    with tile.TileContext(nc) as tc:
        for _ in range(repeat):
            build_block(tc, outs, ins, n, da, db)
    nc.compile()
    return nc


def _f8(a):
    return np.clip(a, -240.0, 240.0).astype(ml_dtypes.float8_e4m3)


def _wlay(w, P_=P):
    # [din, dout] -> [P, din/P, dout], k = kt*P + p
    din, dout = w.shape
    return np.ascontiguousarray(w.reshape(din // P_, P_, dout).transpose(1, 0, 2))


def prepare_maps(inputs, n=N_FULL, da=DA_FULL, db=DB_FULL):
    """Host-side prep: fp8 scaling/layout + exact bias folding."""
    f32 = np.float32
    bf = ml_dtypes.bfloat16
    g = {k: np.ascontiguousarray(np.asarray(v)) for k, v in inputs.items()}
    nb = g["x_a"].shape[0]
    KB = db // P

    b_eff_sa = (g["sa_bv"].astype(f32) @ g["sa_wo"].astype(f32) + g["sa_bo"].astype(f32))
    b_eff_ca = (g["ca_bv"].astype(f32) @ g["ca_wo"].astype(f32) + g["ca_bo"].astype(f32))
    xbpb = (g["x_b"].astype(f32) + b_eff_sa[None, None, :]).astype(bf)

    ones = np.full((P, 2, 16), ONES_VAL, ml_dtypes.float8_e4m3)
    common = {"ones": ones}
    for nm in ("sa_wq", "sa_wk", "sa_wv", "sa_wo", "ca_wq", "ca_wk", "ca_wv", "ca_wo"):
        common[nm] = _f8(_wlay(g[nm].astype(f32) * WS))
    common["bq_sa"] = np.ascontiguousarray(
        (g["sa_bq"].astype(f32) * WS).reshape(KB, P).T)
    common["bq_ca"] = np.ascontiguousarray(
        (g["ca_bq"].astype(f32) * WS).reshape(KB, P).T)

    in_maps = []
    for b in range(nb):
        xbT = _f8(np.ascontiguousarray(
            g["x_b"][b].T.astype(f32).reshape(db // P, P, n).transpose(1, 0, 2)))
        xaT = _f8(np.ascontiguousarray(
            g["x_a"][b].T.astype(f32).reshape(da // P, P, n).transpose(1, 0, 2)))
        in_maps.append(dict(
            xbT=xbT, xaT=xaT, xbpb=np.ascontiguousarray(xbpb[b]), **common,
        ))
    return in_maps, b_eff_ca


_CACHE = {}


def run_on_device(inputs, trace=False, **run_kwargs):
    if not trace:
        os.environ.setdefault("BASS_NEVER_TRACE", "1")
    if "nc" not in _CACHE:
        _CACHE["nc"] = build_program()
    nc = _CACHE["nc"]
    in_maps, add_out = prepare_maps(inputs)
    res = run_bass_kernel_spmd(
        nc, in_maps, core_ids=list(range(len(in_maps))), trace=trace, **run_kwargs,
    )
    out = np.stack([r["out"] for r in res.results], axis=0)
    out = (out + add_out[None, None, :]).astype(np.float32)
    return out, res


def kernel(**inputs) -> np.ndarray:
    out, _ = run_on_device(inputs)
    return out
